# revision 8
# baseline (speedup 1.0000x reference)
"""Multi-head attention kernel for 8 Trainium2 NeuronCores.

Problem: B=4, S=2048, D=1024, H=16, Dh=64 MHA with key-side boolean mask.

Sharding: core c handles (batch b = c//2, head-half g = c%2, 8 heads each).
QKV are column-parallel, the output projection is row-parallel (Megatron
style); the host sums the two partial output projections per batch and adds
the output bias.

Host-side preprocessing (pure data marshalling, exact):
  - x is transposed per batch (matmul on PE needs the contraction dim on
    partitions, so x^T is required for every projection anyway).
  - Keys with mask=False contribute exactly zero after softmax, so the host
    gathers only the unmasked keys (padded to a multiple of 384 with zero
    rows whose exp-bias is -1e30 => exp == 0 exactly). This is exact, not an
    approximation, and cuts score/exp/attn-V work roughly in half.

On-core dataflow (all matmuls in float32r):
  xT --(Wk,Wv stationary/moving)--> KT[f,k], V[k,f] (+biases fused)
  xT --(Wq)--> QT[f,q]
  scores^T[k,q] = KT_h^T-slice x QT_h-slice  (K=64, two heads row-packed)
  E = exp(scores*0.125 + maskbias[k])        (one ScalarE pass, mask fused)
  out_aug[65,q] = [V_h | ones]^T x E         (row 64 = softmax denominator)
  attnT[f,q] = out_aug[0:64] * bcast(1/den)  (K=1 ones matmul broadcast)
  out[s,D] = attnT^T x Wo                    (partial; host adds pair + bo)
"""

import os
import numpy as np

os.environ.setdefault("MYCRO_LOCAL_CACHE", "1")

D_MODEL = 1024
N_HEADS = 16
D_HEAD = 64
BATCH = 4
SEQ = 2048
N_CORES = 8
FH = 512          # features per core (8 heads x 64)
HPC = 8           # heads per core
NEG = -1.0e30     # additive bias for padded/masked keys; exp -> 0 exactly

_COMPILED = {}    # k_pad -> (nc, names)
last_results = None  # BassKernelResults of the most recent run (for test.py)


def _build(k_pad):
    """Emit + compile the per-core bass kernel for a given padded key count."""
    import concourse.bacc as bacc
    import concourse.bass as bass
    import concourse.tile as tile
    from concourse import mybir

    f32 = mybir.dt.float32
    f32r = mybir.dt.float32r
    KT_N = k_pad // 128                     # number of 128-key tiles
    KC = 512 if k_pad % 512 == 0 else 384   # key-side chunk (fp32r needs N>=256)
    assert k_pad % KC == 0 and KC % 128 == 0
    NKC = k_pad // KC

    nc = bacc.Bacc("TRN2", target_bir_lowering=False, debug=False,
                   num_devices=N_CORES)

    dxT = nc.dram_tensor("xT", [D_MODEL, SEQ], f32r, kind="ExternalInput")
    dxkT = nc.dram_tensor("xkT", [D_MODEL, k_pad], f32r, kind="ExternalInput")
    dWq = nc.dram_tensor("Wq", [D_MODEL, FH], f32r, kind="ExternalInput")
    dWk = nc.dram_tensor("Wk", [D_MODEL, FH], f32r, kind="ExternalInput")
    dWv = nc.dram_tensor("Wv", [D_MODEL, FH], f32r, kind="ExternalInput")
    dWo = nc.dram_tensor("Wo", [FH, D_MODEL], f32r, kind="ExternalInput")
    dbq = nc.dram_tensor("bq", [FH], f32, kind="ExternalInput")
    dbk = nc.dram_tensor("bk", [FH], f32, kind="ExternalInput")
    dbv = nc.dram_tensor("bv", [FH], f32r, kind="ExternalInput")
    dmb = nc.dram_tensor("maskb", [k_pad], f32, kind="ExternalInput")
    dones = nc.dram_tensor("ones", [128], f32r, kind="ExternalInput")
    dout = nc.dram_tensor("out", [SEQ, D_MODEL], f32, kind="ExternalOutput")

    EXP = mybir.ActivationFunctionType.Exp

    with tile.TileContext(nc) as tc:
        with tc.tile_pool(name="persist", bufs=1) as pers:
            # ---- weights / constants in SBUF ----
            wo = pers.tile([128, 4, D_MODEL], f32r, tag="wo")
            nc.sync.dma_start(out=wo, in_=dWo.ap().rearrange("(t p) d -> p t d", p=128))
            bq = pers.tile([128, 4], f32, tag="bq")
            nc.sync.dma_start(out=bq, in_=dbq.ap().rearrange("(t p) -> p t", p=128))
            bk = pers.tile([128, 4], f32, tag="bk")
            nc.sync.dma_start(out=bk, in_=dbk.ap().rearrange("(t p) -> p t", p=128))
            bv_row = pers.tile([1, FH], f32r, tag="bvr")
            nc.sync.dma_start(out=bv_row, in_=dbv.ap()[None, :])
            mb = pers.tile([128, KT_N], f32, tag="mb")
            nc.sync.dma_start(out=mb, in_=dmb.ap().rearrange("(t p) -> p t", p=128))
            ones_t = pers.tile([1, 128], f32r, tag="ones")
            nc.sync.dma_start(out=ones_t, in_=dones.ap()[None, :])
            ones64 = ones_t[:, 0:64]
            ones128 = ones_t[:, :]

            # ---- persistent activations ----
            QT = pers.tile([128, 4, SEQ], f32r, tag="QT")       # [f, q]
            KTt = pers.tile([128, 4, k_pad], f32r, tag="KTt")   # [f, k]
            Vau = pers.tile([128, KT_N, HPC, 65], f32r, tag="Vau")  # [k, kt, h, dv+1]
            attnT = pers.tile([128, 4, SEQ], f32r, tag="attnT")  # [f, q]
            ones_bc = bass.AP(tensor=dones.ap().tensor, offset=0,
                              ap=[[0, 128], [0, KT_N * HPC], [1, 1]])
            nc.sync.dma_start(
                out=Vau.rearrange("p a h c -> p (a h) c")[:, :, 64:65],
                in_=ones_bc)

            # ================= K-side projections (KT, V) =================
            with tc.tile_pool(name="xk", bufs=2) as xkp, \
                 tc.tile_pool(name="pk", bufs=6, space="PSUM") as pk:
                wk = xkp.tile([128, 8, FH], f32r, tag="wk")
                nc.sync.dma_start(out=wk, in_=dWk.ap().rearrange("(t p) f -> p t f", p=128))
                wv = xkp.tile([128, 8, FH], f32r, tag="wv")
                nc.sync.dma_start(out=wv, in_=dWv.ap().rearrange("(t p) f -> p t f", p=128))
                xk_r = dxkT.ap().rearrange("(t p) k -> p t k", p=128)
                for kc in range(NKC):
                    xk_t = xkp.tile([128, 8, KC], f32r, tag="xk")
                    nc.sync.dma_start(out=xk_t, in_=xk_r[:, :, kc * KC:(kc + 1) * KC])
                    for ft in range(4):
                        ps = pk.tile([128, KC], f32, tag="pk")
                        for dt in range(8):
                            nc.tensor.matmul(
                                ps,
                                lhsT=wk[:, dt, ft * 128:(ft + 1) * 128],
                                rhs=xk_t[:, dt, :],
                                start=(dt == 0), stop=(dt == 7))
                        nc.vector.tensor_scalar_add(
                            KTt[:, ft, kc * KC:(kc + 1) * KC], ps, bk[:, ft:ft + 1])
                    for kb in range(KC // 128):
                        kg = kc * (KC // 128) + kb
                        ps = pk.tile([128, FH], f32, tag="pk")
                        for dt in range(8):
                            nc.tensor.matmul(
                                ps,
                                lhsT=xk_t[:, dt, kb * 128:(kb + 1) * 128],
                                rhs=wv[:, dt, :],
                                start=(dt == 0), stop=False)
                        nc.tensor.matmul(ps, lhsT=ones128,
                                         rhs=bv_row,
                                         start=False, stop=True)
                        nc.vector.tensor_copy(
                            Vau[:, kg, :, 0:64],
                            ps.rearrange("p (h d) -> p h d", h=HPC))

            # ================= Q-side projection (QT) =================
            with tc.tile_pool(name="xq", bufs=2) as xqp, \
                 tc.tile_pool(name="pq", bufs=6, space="PSUM") as pq:
                wq = xqp.tile([128, 8, FH], f32r, tag="wq")
                nc.sync.dma_start(out=wq, in_=dWq.ap().rearrange("(t p) f -> p t f", p=128))
                xq_r = dxT.ap().rearrange("(t p) s -> p t s", p=128)
                for qc in range(4):
                    xq_t = xqp.tile([128, 8, 512], f32r, tag="xq")
                    nc.sync.dma_start(out=xq_t, in_=xq_r[:, :, qc * 512:(qc + 1) * 512])
                    for ft in range(4):
                        ps = pq.tile([128, 512], f32, tag="pq")
                        for dt in range(8):
                            nc.tensor.matmul(
                                ps,
                                lhsT=wq[:, dt, ft * 128:(ft + 1) * 128],
                                rhs=xq_t[:, dt, :],
                                start=(dt == 0), stop=(dt == 7))
                        nc.vector.tensor_scalar_add(
                            QT[:, ft, qc * 512:(qc + 1) * 512], ps, bq[:, ft:ft + 1])

            # ================= attention core =================
            with tc.tile_pool(name="et", bufs=4) as etp, \
                 tc.tile_pool(name="ua", bufs=2) as uap, \
                 tc.tile_pool(name="rp", bufs=2) as rpp, \
                 tc.tile_pool(name="sp", bufs=2, space="PSUM") as sp, \
                 tc.tile_pool(name="av", bufs=2, space="PSUM") as avp:
                for t in range(4):          # head pair (heads 2t, 2t+1)
                    for qh in range(2):     # query half (1024 queries)
                        q0 = qh * 1024
                        avA = avp.tile([65, 1024], f32, tag="av")
                        avB = avp.tile([65, 1024], f32, tag="av")
                        for kt in range(KT_N):
                            sA = sp.tile([128, 1024], f32, tag="s")
                            sB = sp.tile([128, 1024], f32, tag="s")
                            for h in range(2):
                                c0, c1 = q0 + h * 512, q0 + (h + 1) * 512
                                nc.tensor.matmul(
                                    sA[:, h * 512:(h + 1) * 512],
                                    lhsT=KTt[0:64, t, kt * 128:(kt + 1) * 128],
                                    rhs=QT[0:64, t, c0:c1],
                                    start=True, stop=True)
                                nc.tensor.matmul(
                                    sB[:, h * 512:(h + 1) * 512],
                                    lhsT=KTt[64:128, t, kt * 128:(kt + 1) * 128],
                                    rhs=QT[64:128, t, c0:c1],
                                    start=True, stop=True)
                            eA = etp.tile([128, 1024], f32r, tag="et")
                            nc.scalar.activation(eA, sA, EXP,
                                                 bias=mb[:, kt:kt + 1], scale=0.125)
                            eB = etp.tile([128, 1024], f32r, tag="et")
                            nc.scalar.activation(eB, sB, EXP,
                                                 bias=mb[:, kt:kt + 1], scale=0.125)
                            for h in range(2):
                                cs = slice(h * 512, (h + 1) * 512)
                                nc.tensor.matmul(
                                    avA[:, cs],
                                    lhsT=Vau[:, kt, 2 * t, :],
                                    rhs=eA[:, cs],
                                    start=(kt == 0), stop=(kt == KT_N - 1))
                                nc.tensor.matmul(
                                    avB[:, cs],
                                    lhsT=Vau[:, kt, 2 * t + 1, :],
                                    rhs=eB[:, cs],
                                    start=(kt == 0), stop=(kt == KT_N - 1))
                        # normalize: attnT = out_aug[0:64] * bcast(1/den)
                        rA = rpp.tile([1, 1024], f32r, tag="r")
                        rB = rpp.tile([1, 1024], f32r, tag="r")
                        with nc.allow_low_precision(reason="fp32r matmul operand"):
                            nc.vector.reciprocal(rA, avA[64:65, :])
                            nc.vector.reciprocal(rB, avB[64:65, :])
                        bcA = sp.tile([64, 1024], f32, tag="s")
                        bcB = sp.tile([64, 1024], f32, tag="s")
                        for h in range(2):
                            cs = slice(h * 512, (h + 1) * 512)
                            nc.tensor.matmul(bcA[:, cs], lhsT=ones64,
                                             rhs=rA[:, cs],
                                             start=True, stop=True)
                            nc.tensor.matmul(bcB[:, cs], lhsT=ones64,
                                             rhs=rB[:, cs],
                                             start=True, stop=True)
                        # DVE reads at most one PSUM operand: stage out_aug's
                        # attn rows through SBUF on ScalarE, multiply on DVE.
                        uA = uap.tile([64, 1024], f32, tag="ua")
                        nc.scalar.copy(uA, avA[0:64, :])
                        uB = uap.tile([64, 1024], f32, tag="ua")
                        nc.scalar.copy(uB, avB[0:64, :])
                        nc.vector.tensor_mul(attnT[0:64, t, q0:q0 + 1024],
                                             uA, bcA)
                        nc.vector.tensor_mul(attnT[64:128, t, q0:q0 + 1024],
                                             uB, bcB)

            # ================= output projection (partial) =================
            with tc.tile_pool(name="op", bufs=2, space="PSUM") as opp, \
                 tc.tile_pool(name="ot", bufs=3) as otp:
                for st in range(16):
                    ps = opp.tile([128, D_MODEL], f32, tag="op")
                    for ft in range(4):
                        for dh in range(2):
                            nc.tensor.matmul(
                                ps[:, dh * 512:(dh + 1) * 512],
                                lhsT=attnT[:, ft, st * 128:(st + 1) * 128],
                                rhs=wo[:, ft, dh * 512:(dh + 1) * 512],
                                start=(ft == 0), stop=(ft == 3))
                    ot = otp.tile([128, D_MODEL], f32, tag="ot")
                    nc.vector.tensor_copy(ot, ps)
                    nc.sync.dma_start(out=dout.ap()[st * 128:(st + 1) * 128, :], in_=ot)

    nc.compile()
    return nc


def _get_compiled(k_pad):
    if k_pad not in _COMPILED:
        _COMPILED[k_pad] = _build(k_pad)
    return _COMPILED[k_pad]


def _prep_core_inputs(x, attention_mask, Wq, bq, Wk, bk, Wv, bv, Wo):
    """Host-side shard prep. Returns (in_maps, k_pad)."""
    x = np.asarray(x, np.float32)
    mask = np.asarray(attention_mask, bool)
    idxs = [np.nonzero(mask[b])[0] for b in range(BATCH)]
    ke_max = max(1, max(len(i) for i in idxs))
    k_pad = 384 * ((ke_max + 383) // 384)
    if k_pad > SEQ:
        k_pad = SEQ

    in_maps = []
    for b in range(BATCH):
        xT = np.ascontiguousarray(x[b].T)            # [D, S]
        idx = idxs[b]
        ke = len(idx)
        if ke > k_pad:  # shouldn't happen (k_pad >= ke_max) unless clipped
            idx = idx[:k_pad]
            ke = k_pad
        xkT = np.zeros((D_MODEL, k_pad), np.float32)
        xkT[:, :ke] = x[b][idx].T
        maskb = np.zeros(k_pad, np.float32)
        maskb[ke:] = NEG
        for g in range(2):
            fs = slice(g * FH, (g + 1) * FH)
            in_maps.append({
                "xT": xT,
                "xkT": xkT,
                "Wq": np.ascontiguousarray(Wq[:, fs], dtype=np.float32),
                "Wk": np.ascontiguousarray(Wk[:, fs], dtype=np.float32),
                "Wv": np.ascontiguousarray(Wv[:, fs], dtype=np.float32),
                "Wo": np.ascontiguousarray(Wo[fs, :], dtype=np.float32),
                "bq": np.ascontiguousarray(bq[fs], dtype=np.float32),
                "bk": np.ascontiguousarray(bk[fs], dtype=np.float32),
                "bv": np.ascontiguousarray(bv[fs], dtype=np.float32),
                "maskb": maskb,
                "ones": np.ones(128, np.float32),
            })
    return in_maps, k_pad


def kernel(x, attention_mask, Wq, bq, Wk, bk, Wv, bv, Wo, bo):
    global last_results
    from concourse.bass_utils import run_bass_kernel_spmd

    in_maps, k_pad = _prep_core_inputs(x, attention_mask, Wq, bq, Wk, bk, Wv, bv, Wo)
    nc = _get_compiled(k_pad)
    res = run_bass_kernel_spmd(nc, in_maps, core_ids=list(range(N_CORES)))
    last_results = res

    bo = np.asarray(bo, np.float32)
    out = np.empty((BATCH, SEQ, D_MODEL), np.float32)
    for b in range(BATCH):
        out[b] = res.results[2 * b]["out"] + res.results[2 * b + 1]["out"] + bo
    return out


# revision 11
# speedup vs baseline: 1.0883x; 1.0883x over previous
"""Multi-head attention kernel for 8 Trainium2 NeuronCores.

Problem: B=4, S=2048, D=1024, H=16, Dh=64 MHA with key-side boolean mask.

Sharding: core c handles (batch b = c//2, head-half g = c%2, 8 heads each).
QKV are column-parallel, the output projection is row-parallel (Megatron
style); the host sums the two partial output projections per batch and adds
the output bias.

Host-side preprocessing (pure data marshalling, exact):
  - All inputs are pre-tiled into DMA-native layouts (partition-major,
    contiguous per partition) so each dma_start lowers to large linear
    descriptors instead of thousands of 2KB strided reads.
  - x is transposed per batch (the PE contracts over the partition dim, so
    x^T is required for every projection).
  - Keys with mask=False contribute exactly zero after softmax, so the host
    gathers only the unmasked keys (padded to a multiple of 384 with zero
    rows whose exp-bias is -1e30 => exp == 0 exactly). Exact, and cuts
    score/exp/attn-V work roughly in half.

On-core dataflow (all matmuls in float32r):
  xT --(Wk,Wv)--> KT[f,k] (zero-padded per head to K=128), V[k,f] (+biases)
  xT --(Wq)--> QT[f,q]
  scores^T[k,q] = [KT_h ; 0]^T x QT_pair   (K=128 full array; the zero rows
                                            kill the other head's features)
  E = exp(scores*0.125 + maskbias[k])      (one ScalarE pass, mask fused)
  out_aug[65,q] = [V_h | ones]^T x E       (row 64 = softmax denominator)
  attnT[f,q] = out_aug[0:64] * bcast(1/den)  (K=1 ones matmul broadcast)
  out[s,D] = attnT^T x Wo                  (partial; host adds pair + bo)
"""

import os
import numpy as np

os.environ.setdefault("MYCRO_LOCAL_CACHE", "1")

D_MODEL = 1024
N_HEADS = 16
D_HEAD = 64
BATCH = 4
SEQ = 2048
N_CORES = 8
FH = 512          # features per core (8 heads x 64)
HPC = 8           # heads per core
NEG = -1.0e30     # additive bias for padded/masked keys; exp -> 0 exactly

_COMPILED = {}    # k_pad -> nc
last_results = None  # BassKernelResults of the most recent run (for test.py)


def _build(k_pad):
    """Emit + compile the per-core bass kernel for a given padded key count."""
    import concourse.bacc as bacc
    import concourse.bass as bass
    import concourse.tile as tile
    from concourse import mybir

    f32 = mybir.dt.float32
    f32r = mybir.dt.float32r
    KT_N = k_pad // 128                     # number of 128-key tiles
    KC = 512 if k_pad % 512 == 0 else 384   # key-side chunk (fp32r needs N>=256)
    assert k_pad % KC == 0 and KC % 128 == 0
    NKC = k_pad // KC

    nc = bacc.Bacc("TRN2", target_bir_lowering=False, debug=False,
                   num_devices=N_CORES)

    # all pre-tiled on host into DMA-native layouts
    dxq = nc.dram_tensor("xq", [4, 128, 8, 512], f32r, kind="ExternalInput")
    dxk = nc.dram_tensor("xk", [NKC, 128, 8, KC], f32r, kind="ExternalInput")
    dWq = nc.dram_tensor("Wq", [128, 8, FH], f32r, kind="ExternalInput")
    dWk = nc.dram_tensor("Wk", [128, 8, FH], f32r, kind="ExternalInput")
    dWv = nc.dram_tensor("Wv", [128, 8, FH], f32r, kind="ExternalInput")
    dWo = nc.dram_tensor("Wo", [128, 4, D_MODEL], f32r, kind="ExternalInput")
    dbq = nc.dram_tensor("bq", [128, 4], f32, kind="ExternalInput")
    dbk = nc.dram_tensor("bk", [128, 4], f32, kind="ExternalInput")
    dbv = nc.dram_tensor("bv", [FH], f32r, kind="ExternalInput")
    dmb = nc.dram_tensor("maskb", [128, KT_N], f32, kind="ExternalInput")
    dcst = nc.dram_tensor("consts", [256], f32r, kind="ExternalInput")  # ones|zeros
    dout = nc.dram_tensor("out", [SEQ, D_MODEL], f32, kind="ExternalOutput")

    EXP = mybir.ActivationFunctionType.Exp
    IDn = mybir.ActivationFunctionType.Identity

    with tile.TileContext(nc) as tc:
        with tc.tile_pool(name="persist", bufs=1) as pers:
            # ---- constants in SBUF ----
            wo = pers.tile([128, 4, D_MODEL], f32r, tag="wo")
            nc.sync.dma_start(out=wo, in_=dWo.ap())
            bq = pers.tile([128, 4], f32, tag="bq")
            nc.sync.dma_start(out=bq, in_=dbq.ap())
            bk = pers.tile([128, 4], f32, tag="bk")
            nc.sync.dma_start(out=bk, in_=dbk.ap())
            bv_row = pers.tile([1, FH], f32r, tag="bvr")
            nc.sync.dma_start(out=bv_row, in_=dbv.ap()[None, :])
            mb = pers.tile([128, KT_N], f32, tag="mb")
            nc.sync.dma_start(out=mb, in_=dmb.ap())
            ones_t = pers.tile([1, 128], f32r, tag="ones")
            nc.sync.dma_start(out=ones_t, in_=dcst.ap()[None, 0:128])
            ones64 = ones_t[:, 0:64]
            ones128 = ones_t[:, :]

            # ---- persistent activations ----
            QT = pers.tile([128, 4, SEQ], f32r, tag="QT")        # [f, q]
            # zero-padded per-head score weights: KTe rows 0:64 = even head,
            # rows 64:128 = 0; KTo rows 0:64 = 0, rows 64:128 = odd head.
            KTe = pers.tile([128, 4, k_pad], f32r, tag="KTe")
            KTo = pers.tile([128, 4, k_pad], f32r, tag="KTo")
            Vau = pers.tile([128, KT_N, HPC, 65], f32r, tag="Vau")
            attnT = pers.tile([128, 4, SEQ], f32r, tag="attnT")  # [f, q]

            zsrc = dcst.ap().tensor
            nz = 4 * k_pad // 128
            zin = bass.AP(tensor=zsrc, offset=128,
                          ap=[[0, 64], [0, nz], [1, 128]])
            nc.sync.dma_start(
                out=KTe[64:128, :, :].rearrange("p a (m n) -> p (a m) n", n=128),
                in_=zin)
            nc.sync.dma_start(
                out=KTo[0:64, :, :].rearrange("p a (m n) -> p (a m) n", n=128),
                in_=zin)
            nc.sync.dma_start(
                out=Vau.rearrange("p a h c -> p (a h) c")[:, :, 64:65],
                in_=bass.AP(tensor=zsrc, offset=0,
                            ap=[[0, 128], [0, KT_N * HPC], [1, 1]]))

            # ================= K-side projections (KT, V) =================
            with tc.tile_pool(name="wtk", bufs=1) as wtk, \
                 tc.tile_pool(name="xk", bufs=2) as xkp, \
                 tc.tile_pool(name="pk", bufs=6, space="PSUM") as pk:
                wk = wtk.tile([128, 8, FH], f32r, tag="wk")
                nc.sync.dma_start(out=wk, in_=dWk.ap())
                wv = wtk.tile([128, 8, FH], f32r, tag="wv")
                nc.sync.dma_start(out=wv, in_=dWv.ap())
                for kc in range(NKC):
                    xk_t = xkp.tile([128, 8, KC], f32r, tag="xk")
                    nc.sync.dma_start(out=xk_t, in_=dxk.ap()[kc])
                    for ft in range(4):
                        ps = pk.tile([128, KC], f32, tag="pk")
                        for dt in range(8):
                            nc.tensor.matmul(
                                ps,
                                lhsT=wk[:, dt, ft * 128:(ft + 1) * 128],
                                rhs=xk_t[:, dt, :],
                                start=(dt == 0), stop=(dt == 7))
                        ks = slice(kc * KC, (kc + 1) * KC)
                        nc.scalar.activation(KTe[0:64, ft, ks], ps[0:64, :],
                                             IDn, bias=bk[0:64, ft:ft + 1])
                        nc.scalar.activation(KTo[64:128, ft, ks], ps[64:128, :],
                                             IDn, bias=bk[64:128, ft:ft + 1])
                    for kb in range(KC // 128):
                        kg = kc * (KC // 128) + kb
                        ps = pk.tile([128, FH], f32, tag="pk")
                        for dt in range(8):
                            nc.tensor.matmul(
                                ps,
                                lhsT=xk_t[:, dt, kb * 128:(kb + 1) * 128],
                                rhs=wv[:, dt, :],
                                start=(dt == 0), stop=False)
                        nc.tensor.matmul(ps, lhsT=ones128, rhs=bv_row,
                                         start=False, stop=True)
                        nc.scalar.copy(
                            Vau[:, kg, :, 0:64],
                            ps.rearrange("p (h d) -> p h d", h=HPC))

            # ================= Q-side projection (QT) =================
            with tc.tile_pool(name="wtq", bufs=1) as wtq, \
                 tc.tile_pool(name="xq", bufs=2) as xqp, \
                 tc.tile_pool(name="pq", bufs=6, space="PSUM") as pq:
                wq = wtq.tile([128, 8, FH], f32r, tag="wq")
                nc.sync.dma_start(out=wq, in_=dWq.ap())
                for qc in range(4):
                    xq_t = xqp.tile([128, 8, 512], f32r, tag="xq")
                    nc.sync.dma_start(out=xq_t, in_=dxq.ap()[qc])
                    for ft in range(4):
                        ps = pq.tile([128, 512], f32, tag="pq")
                        for dt in range(8):
                            nc.tensor.matmul(
                                ps,
                                lhsT=wq[:, dt, ft * 128:(ft + 1) * 128],
                                rhs=xq_t[:, dt, :],
                                start=(dt == 0), stop=(dt == 7))
                        nc.scalar.activation(QT[:, ft, qc * 512:(qc + 1) * 512],
                                             ps, IDn, bias=bq[:, ft:ft + 1])

            # ================= attention core =================
            with tc.tile_pool(name="et", bufs=4) as etp, \
                 tc.tile_pool(name="ua", bufs=2) as uap, \
                 tc.tile_pool(name="rp", bufs=2) as rpp, \
                 tc.tile_pool(name="sp", bufs=2, space="PSUM") as sp, \
                 tc.tile_pool(name="av", bufs=2, space="PSUM") as avp:
                for t in range(4):          # head pair (heads 2t, 2t+1)
                    for qh in range(2):     # query half (1024 queries)
                        q0 = qh * 1024
                        avA = avp.tile([65, 1024], f32, tag="av")
                        avB = avp.tile([65, 1024], f32, tag="av")
                        for kt in range(KT_N):
                            kts = slice(kt * 128, (kt + 1) * 128)
                            sA = sp.tile([128, 1024], f32, tag="s")
                            sB = sp.tile([128, 1024], f32, tag="s")
                            for h in range(2):
                                c0, c1 = q0 + h * 512, q0 + (h + 1) * 512
                                nc.tensor.matmul(
                                    sA[:, h * 512:(h + 1) * 512],
                                    lhsT=KTe[:, t, kts], rhs=QT[:, t, c0:c1],
                                    start=True, stop=True)
                                nc.tensor.matmul(
                                    sB[:, h * 512:(h + 1) * 512],
                                    lhsT=KTo[:, t, kts], rhs=QT[:, t, c0:c1],
                                    start=True, stop=True)
                            eA = etp.tile([128, 1024], f32r, tag="et")
                            nc.scalar.activation(eA, sA, EXP,
                                                 bias=mb[:, kt:kt + 1], scale=0.125)
                            eB = etp.tile([128, 1024], f32r, tag="et")
                            nc.scalar.activation(eB, sB, EXP,
                                                 bias=mb[:, kt:kt + 1], scale=0.125)
                            for h in range(2):
                                cs = slice(h * 512, (h + 1) * 512)
                                nc.tensor.matmul(
                                    avA[:, cs], lhsT=Vau[:, kt, 2 * t, :],
                                    rhs=eA[:, cs],
                                    start=(kt == 0), stop=(kt == KT_N - 1))
                                nc.tensor.matmul(
                                    avB[:, cs], lhsT=Vau[:, kt, 2 * t + 1, :],
                                    rhs=eB[:, cs],
                                    start=(kt == 0), stop=(kt == KT_N - 1))
                        # normalize: attnT = out_aug[0:64] * bcast(1/den)
                        rA = rpp.tile([1, 1024], f32r, tag="r")
                        rB = rpp.tile([1, 1024], f32r, tag="r")
                        with nc.allow_low_precision(reason="fp32r matmul operand"):
                            nc.vector.reciprocal(rA, avA[64:65, :])
                            nc.vector.reciprocal(rB, avB[64:65, :])
                        bcA = sp.tile([64, 1024], f32, tag="s")
                        bcB = sp.tile([64, 1024], f32, tag="s")
                        for h in range(2):
                            cs = slice(h * 512, (h + 1) * 512)
                            nc.tensor.matmul(bcA[:, cs], lhsT=ones64,
                                             rhs=rA[:, cs], start=True, stop=True)
                            nc.tensor.matmul(bcB[:, cs], lhsT=ones64,
                                             rhs=rB[:, cs], start=True, stop=True)
                        # DVE reads at most one PSUM operand: stage out_aug's
                        # attn rows through SBUF, then multiply.
                        uA = uap.tile([64, 1024], f32, tag="ua")
                        nc.vector.tensor_copy(uA, avA[0:64, :])
                        uB = uap.tile([64, 1024], f32, tag="ua")
                        nc.vector.tensor_copy(uB, avB[0:64, :])
                        nc.vector.tensor_mul(attnT[0:64, t, q0:q0 + 1024],
                                             uA, bcA)
                        nc.vector.tensor_mul(attnT[64:128, t, q0:q0 + 1024],
                                             uB, bcB)

            # ================= output projection (partial) =================
            with tc.tile_pool(name="op", bufs=2, space="PSUM") as opp, \
                 tc.tile_pool(name="ot", bufs=3) as otp:
                for st in range(16):
                    ps = opp.tile([128, D_MODEL], f32, tag="op")
                    for ft in range(4):
                        for dh in range(2):
                            nc.tensor.matmul(
                                ps[:, dh * 512:(dh + 1) * 512],
                                lhsT=attnT[:, ft, st * 128:(st + 1) * 128],
                                rhs=wo[:, ft, dh * 512:(dh + 1) * 512],
                                start=(ft == 0), stop=(ft == 3))
                    ot = otp.tile([128, D_MODEL], f32, tag="ot")
                    nc.scalar.copy(ot, ps)
                    nc.sync.dma_start(out=dout.ap()[st * 128:(st + 1) * 128, :], in_=ot)

    nc.compile()
    return nc


def _get_compiled(k_pad):
    if k_pad not in _COMPILED:
        _COMPILED[k_pad] = _build(k_pad)
    return _COMPILED[k_pad]


def _tile_pf(a, p=128):
    """[P*t, f...] -> contiguous [p, t, f...] partition-major tiling."""
    t = a.shape[0] // p
    return np.ascontiguousarray(
        a.reshape(t, p, *a.shape[1:]).swapaxes(0, 1))


def _prep_core_inputs(x, attention_mask, Wq, bq, Wk, bk, Wv, bv, Wo):
    """Host-side shard prep. Returns (in_maps, k_pad)."""
    x = np.asarray(x, np.float32)
    mask = np.asarray(attention_mask, bool)
    idxs = [np.nonzero(mask[b])[0] for b in range(BATCH)]
    ke_max = max(1, max(len(i) for i in idxs))
    k_pad = 384 * ((ke_max + 383) // 384)
    if k_pad > SEQ:
        k_pad = SEQ
    KC = 512 if k_pad % 512 == 0 else 384
    NKC = k_pad // KC
    KT_N = k_pad // 128

    consts = np.zeros(256, np.float32)
    consts[0:128] = 1.0

    in_maps = []
    for b in range(BATCH):
        xT = x[b].T                                  # [D, S] view
        # xq: [qc, p, dt, 512]
        xq = np.ascontiguousarray(
            xT.reshape(8, 128, 4, 512).transpose(2, 1, 0, 3))
        idx = idxs[b]
        ke = len(idx)
        if ke > k_pad:
            idx = idx[:k_pad]
            ke = k_pad
        xkT = np.zeros((D_MODEL, k_pad), np.float32)
        xkT[:, :ke] = x[b][idx].T
        # xk: [kc, p, dt, KC]
        xk = np.ascontiguousarray(
            xkT.reshape(8, 128, NKC, KC).transpose(2, 1, 0, 3))
        maskb = np.zeros(k_pad, np.float32)
        maskb[ke:] = NEG
        mb_t = _tile_pf(maskb)                       # [128, KT_N]
        for g in range(2):
            fs = slice(g * FH, (g + 1) * FH)
            in_maps.append({
                "xq": xq,
                "xk": xk,
                "Wq": _tile_pf(np.asarray(Wq[:, fs], np.float32)),
                "Wk": _tile_pf(np.asarray(Wk[:, fs], np.float32)),
                "Wv": _tile_pf(np.asarray(Wv[:, fs], np.float32)),
                "Wo": _tile_pf(np.asarray(Wo[fs, :], np.float32)),
                "bq": _tile_pf(np.asarray(bq[fs], np.float32)),
                "bk": _tile_pf(np.asarray(bk[fs], np.float32)),
                "bv": np.ascontiguousarray(bv[fs]).astype(np.float32),
                "maskb": mb_t,
                "consts": consts,
            })
    return in_maps, k_pad


def kernel(x, attention_mask, Wq, bq, Wk, bk, Wv, bv, Wo, bo):
    global last_results
    from concourse.bass_utils import run_bass_kernel_spmd

    in_maps, k_pad = _prep_core_inputs(x, attention_mask, Wq, bq, Wk, bk, Wv, bv, Wo)
    nc = _get_compiled(k_pad)
    res = run_bass_kernel_spmd(nc, in_maps, core_ids=list(range(N_CORES)))
    last_results = res

    bo = np.asarray(bo, np.float32)
    out = np.empty((BATCH, SEQ, D_MODEL), np.float32)
    for b in range(BATCH):
        out[b] = res.results[2 * b]["out"] + res.results[2 * b + 1]["out"] + bo
    return out


# revision 13
# speedup vs baseline: 1.2326x; 1.1326x over previous
"""Multi-head attention kernel for 8 Trainium2 NeuronCores.

Problem: B=4, S=2048, D=1024, H=16, Dh=64 MHA with key-side boolean mask.

Sharding: core c handles (batch b = c//2, head-half g = c%2, 8 heads each).
QKV are column-parallel, the output projection is row-parallel (Megatron
style); the host sums the two partial output projections per batch and adds
the output bias.

Host-side preprocessing (pure data marshalling, exact):
  - All inputs are pre-tiled into DMA-native layouts (partition-major,
    contiguous per partition) so each dma_start lowers to large linear
    descriptors instead of thousands of 2KB strided reads.
  - x is transposed per batch (the PE contracts over the partition dim, so
    x^T is required for every projection).
  - Keys with mask=False contribute exactly zero after softmax, so the host
    gathers only the unmasked keys (padded to a multiple of 384 with zero
    rows whose exp-bias is -1e30 => exp == 0 exactly). Exact, and cuts
    score/exp/attn-V work roughly in half.

On-core dataflow (all matmuls in float32r):
  xT --(Wk,Wv)--> KT[f,k] (zero-padded per head to K=128), V[k,f] (+biases)
  xT --(Wq)--> QT[f,q]
  scores^T[k,q] = [KT_h ; 0]^T x QT_pair   (K=128 full array; the zero rows
                                            kill the other head's features)
  E = exp(scores*0.125 + maskbias[k])      (one ScalarE pass, mask fused)
  out_aug[65,q] = [V_h | ones]^T x E       (row 64 = softmax denominator)
  attnT[f,q] = out_aug[0:64] * bcast(1/den)  (K=1 ones matmul broadcast)
  out[s,D] = attnT^T x Wo                  (partial; host adds pair + bo)
"""

import os
import numpy as np

os.environ.setdefault("MYCRO_LOCAL_CACHE", "1")

D_MODEL = 1024
N_HEADS = 16
D_HEAD = 64
BATCH = 4
SEQ = 2048
N_CORES = 8
FH = 512          # features per core (8 heads x 64)
HPC = 8           # heads per core
NEG = -1.0e30     # additive bias for padded/masked keys; exp -> 0 exactly

_COMPILED = {}    # k_pad -> nc
last_results = None  # BassKernelResults of the most recent run (for test.py)


def _build(k_pad):
    """Emit + compile the per-core bass kernel for a given padded key count."""
    import concourse.bacc as bacc
    import concourse.bass as bass
    import concourse.tile as tile
    from concourse import mybir

    f32 = mybir.dt.float32
    f32r = mybir.dt.float32r
    KT_N = k_pad // 128                     # number of 128-key tiles
    KC = 512 if k_pad % 512 == 0 else 384   # key-side chunk (fp32r needs N>=256)
    assert k_pad % KC == 0 and KC % 128 == 0
    NKC = k_pad // KC

    nc = bacc.Bacc("TRN2", target_bir_lowering=False, debug=False,
                   num_devices=N_CORES)

    # all pre-tiled on host into DMA-native layouts
    dxq = nc.dram_tensor("xq", [4, 128, 8, 512], f32r, kind="ExternalInput")
    dxk = nc.dram_tensor("xk", [NKC, 128, 8, KC], f32r, kind="ExternalInput")
    dWq = nc.dram_tensor("Wq", [128, 8, FH], f32r, kind="ExternalInput")
    dWk = nc.dram_tensor("Wk", [128, 8, FH], f32r, kind="ExternalInput")
    dWv = nc.dram_tensor("Wv", [128, 8, HPC * 65], f32r, kind="ExternalInput")
    dWo = nc.dram_tensor("Wo", [128, 4, D_MODEL], f32r, kind="ExternalInput")
    dbq = nc.dram_tensor("bq", [128, 4], f32, kind="ExternalInput")
    dbk = nc.dram_tensor("bk", [128, 4], f32, kind="ExternalInput")
    dbv = nc.dram_tensor("bv", [HPC * 65], f32r, kind="ExternalInput")
    dmb = nc.dram_tensor("maskb", [128, KT_N], f32, kind="ExternalInput")
    dcst = nc.dram_tensor("consts", [256], f32r, kind="ExternalInput")  # ones|zeros
    dzp = nc.dram_tensor("zpad", [4 * k_pad], f32r, kind="ExternalInput")
    dout = nc.dram_tensor("out", [SEQ, D_MODEL], f32, kind="ExternalOutput")

    EXP = mybir.ActivationFunctionType.Exp
    IDn = mybir.ActivationFunctionType.Identity

    with tile.TileContext(nc) as tc:
        with tc.tile_pool(name="persist", bufs=1) as pers:
            # ---- constants in SBUF ----
            wo = pers.tile([128, 4, D_MODEL], f32r, tag="wo")
            nc.sync.dma_start(out=wo, in_=dWo.ap())
            bq = pers.tile([128, 4], f32, tag="bq")
            nc.sync.dma_start(out=bq, in_=dbq.ap())
            bk = pers.tile([128, 4], f32, tag="bk")
            nc.sync.dma_start(out=bk, in_=dbk.ap())
            bv_row = pers.tile([1, HPC * 65], f32r, tag="bvr")
            nc.sync.dma_start(out=bv_row, in_=dbv.ap()[None, :])
            mb = pers.tile([128, KT_N], f32, tag="mb")
            nc.sync.dma_start(out=mb, in_=dmb.ap())
            ones_t = pers.tile([1, 128], f32r, tag="ones")
            nc.sync.dma_start(out=ones_t, in_=dcst.ap()[None, 0:128])
            ones64 = ones_t[:, 0:64]
            ones128 = ones_t[:, :]

            # ---- persistent activations ----
            QT = pers.tile([128, 4, SEQ], f32r, tag="QT")        # [f, q]
            # zero-padded per-head score weights: KTe rows 0:64 = even head,
            # rows 64:128 = 0; KTo rows 0:64 = 0, rows 64:128 = odd head.
            KTe = pers.tile([128, 4, k_pad], f32r, tag="KTe")
            KTo = pers.tile([128, 4, k_pad], f32r, tag="KTo")
            Vau = pers.tile([128, KT_N, HPC, 65], f32r, tag="Vau")
            attnT = pers.tile([128, 4, SEQ], f32r, tag="attnT")  # [f, q]

            zin = bass.AP(tensor=dzp.ap().tensor, offset=0,
                          ap=[[0, 64], [1, 4 * k_pad]])
            nc.sync.dma_start(
                out=KTo[0:64, :, :].rearrange("p a k -> p (a k)"), in_=zin)

            # ================= K-side projections (KT, V) =================
            with tc.tile_pool(name="wtk", bufs=1) as wtk, \
                 tc.tile_pool(name="xk", bufs=2) as xkp, \
                 tc.tile_pool(name="pk", bufs=4, space="PSUM") as pk:
                wk = wtk.tile([128, 8, FH], f32r, tag="wk")
                nc.sync.dma_start(out=wk, in_=dWk.ap())
                wv = wtk.tile([128, 8, HPC * 65], f32r, tag="wv")
                nc.sync.dma_start(out=wv, in_=dWv.ap())
                for kc in range(NKC):
                    xk_t = xkp.tile([128, 8, KC], f32r, tag="xk")
                    nc.sync.dma_start(out=xk_t, in_=dxk.ap()[kc])
                    for ft in range(4):
                        ps = pk.tile([128, KC], f32, tag="pk")
                        for dt in range(8):
                            nc.tensor.matmul(
                                ps,
                                lhsT=wk[:, dt, ft * 128:(ft + 1) * 128],
                                rhs=xk_t[:, dt, :],
                                start=(dt == 0), stop=(dt == 7))
                        ks = slice(kc * KC, (kc + 1) * KC)
                        nc.scalar.activation(KTe[:, ft, ks], ps, IDn,
                                             bias=bk[:, ft:ft + 1])
                    for kb in range(KC // 128):
                        kg = kc * (KC // 128) + kb
                        ps = pk.tile([128, HPC * 65], f32, tag="pk")
                        for dt in range(8):
                            nc.tensor.matmul(
                                ps[:, 0:512],
                                lhsT=xk_t[:, dt, kb * 128:(kb + 1) * 128],
                                rhs=wv[:, dt, 0:512],
                                start=(dt == 0), stop=False)
                            nc.tensor.matmul(
                                ps[:, 512:520],
                                lhsT=xk_t[:, dt, kb * 128:(kb + 1) * 128],
                                rhs=wv[:, dt, 512:520],
                                start=(dt == 0), stop=False)
                        nc.tensor.matmul(ps[:, 0:512], lhsT=ones128,
                                         rhs=bv_row[:, 0:512],
                                         start=False, stop=True)
                        nc.tensor.matmul(ps[:, 512:520], lhsT=ones128,
                                         rhs=bv_row[:, 512:520],
                                         start=False, stop=True)
                        nc.scalar.copy(Vau[:, kg, :, :], ps)

            # finish zero-padded score weights: KTo upper half is a copy of
            # the drained KTe upper half; then KTe upper half becomes zero.
            nc.vector.tensor_copy(
                KTo[64:128, :, :].rearrange("p a k -> p (a k)"),
                KTe[64:128, :, :].rearrange("p a k -> p (a k)"))
            nc.sync.dma_start(
                out=KTe[64:128, :, :].rearrange("p a k -> p (a k)"), in_=zin)

            # ================= Q-side projection (QT) =================
            with tc.tile_pool(name="wtq", bufs=1) as wtq, \
                 tc.tile_pool(name="xq", bufs=2) as xqp, \
                 tc.tile_pool(name="pq", bufs=6, space="PSUM") as pq:
                wq = wtq.tile([128, 8, FH], f32r, tag="wq")
                nc.sync.dma_start(out=wq, in_=dWq.ap())
                for qc in range(4):
                    xq_t = xqp.tile([128, 8, 512], f32r, tag="xq")
                    nc.sync.dma_start(out=xq_t, in_=dxq.ap()[qc])
                    for ft in range(4):
                        ps = pq.tile([128, 512], f32, tag="pq")
                        for dt in range(8):
                            nc.tensor.matmul(
                                ps,
                                lhsT=wq[:, dt, ft * 128:(ft + 1) * 128],
                                rhs=xq_t[:, dt, :],
                                start=(dt == 0), stop=(dt == 7))
                        nc.scalar.activation(QT[:, ft, qc * 512:(qc + 1) * 512],
                                             ps, IDn, bias=bq[:, ft:ft + 1])

            # ================= attention core =================
            with tc.tile_pool(name="et", bufs=4) as etp, \
                 tc.tile_pool(name="ua", bufs=2) as uap, \
                 tc.tile_pool(name="rp", bufs=2) as rpp, \
                 tc.tile_pool(name="sp", bufs=2, space="PSUM") as sp, \
                 tc.tile_pool(name="av", bufs=2, space="PSUM") as avp:
                for t in range(4):          # head pair (heads 2t, 2t+1)
                    for qh in range(2):     # query half (1024 queries)
                        q0 = qh * 1024
                        avA = avp.tile([65, 1024], f32, tag="av")
                        avB = avp.tile([65, 1024], f32, tag="av")
                        for kt in range(KT_N):
                            kts = slice(kt * 128, (kt + 1) * 128)
                            sA = sp.tile([128, 1024], f32, tag="s")
                            sB = sp.tile([128, 1024], f32, tag="s")
                            for h in range(2):
                                c0, c1 = q0 + h * 512, q0 + (h + 1) * 512
                                nc.tensor.matmul(
                                    sA[:, h * 512:(h + 1) * 512],
                                    lhsT=KTe[:, t, kts], rhs=QT[:, t, c0:c1],
                                    start=True, stop=True)
                                nc.tensor.matmul(
                                    sB[:, h * 512:(h + 1) * 512],
                                    lhsT=KTo[:, t, kts], rhs=QT[:, t, c0:c1],
                                    start=True, stop=True)
                            eA = etp.tile([128, 1024], f32r, tag="et")
                            nc.scalar.activation(eA, sA, EXP,
                                                 bias=mb[:, kt:kt + 1], scale=0.125)
                            eB = etp.tile([128, 1024], f32r, tag="et")
                            nc.scalar.activation(eB, sB, EXP,
                                                 bias=mb[:, kt:kt + 1], scale=0.125)
                            for h in range(2):
                                cs = slice(h * 512, (h + 1) * 512)
                                nc.tensor.matmul(
                                    avA[:, cs], lhsT=Vau[:, kt, 2 * t, :],
                                    rhs=eA[:, cs],
                                    start=(kt == 0), stop=(kt == KT_N - 1))
                                nc.tensor.matmul(
                                    avB[:, cs], lhsT=Vau[:, kt, 2 * t + 1, :],
                                    rhs=eB[:, cs],
                                    start=(kt == 0), stop=(kt == KT_N - 1))
                        # normalize: attnT = out_aug[0:64] * bcast(1/den)
                        rA = rpp.tile([1, 1024], f32r, tag="r")
                        rB = rpp.tile([1, 1024], f32r, tag="r")
                        with nc.allow_low_precision(reason="fp32r matmul operand"):
                            nc.vector.reciprocal(rA, avA[64:65, :])
                            nc.vector.reciprocal(rB, avB[64:65, :])
                        bcA = sp.tile([64, 1024], f32, tag="s")
                        bcB = sp.tile([64, 1024], f32, tag="s")
                        for h in range(2):
                            cs = slice(h * 512, (h + 1) * 512)
                            nc.tensor.matmul(bcA[:, cs], lhsT=ones64,
                                             rhs=rA[:, cs], start=True, stop=True)
                            nc.tensor.matmul(bcB[:, cs], lhsT=ones64,
                                             rhs=rB[:, cs], start=True, stop=True)
                        # DVE reads at most one PSUM operand: stage out_aug's
                        # attn rows through SBUF, then multiply.
                        uA = uap.tile([64, 1024], f32, tag="ua")
                        nc.scalar.copy(uA, avA[0:64, :])
                        uB = uap.tile([64, 1024], f32, tag="ua")
                        nc.scalar.copy(uB, avB[0:64, :])
                        nc.vector.tensor_mul(attnT[0:64, t, q0:q0 + 1024],
                                             uA, bcA)
                        nc.vector.tensor_mul(attnT[64:128, t, q0:q0 + 1024],
                                             uB, bcB)

            # ================= output projection (partial) =================
            with tc.tile_pool(name="op", bufs=2, space="PSUM") as opp, \
                 tc.tile_pool(name="ot", bufs=3) as otp:
                for st in range(16):
                    ps = opp.tile([128, D_MODEL], f32, tag="op")
                    for ft in range(4):
                        for dh in range(2):
                            nc.tensor.matmul(
                                ps[:, dh * 512:(dh + 1) * 512],
                                lhsT=attnT[:, ft, st * 128:(st + 1) * 128],
                                rhs=wo[:, ft, dh * 512:(dh + 1) * 512],
                                start=(ft == 0), stop=(ft == 3))
                    ot = otp.tile([128, D_MODEL], f32, tag="ot")
                    nc.scalar.copy(ot, ps)
                    nc.sync.dma_start(out=dout.ap()[st * 128:(st + 1) * 128, :], in_=ot)

    nc.compile()
    return nc


def _get_compiled(k_pad):
    if k_pad not in _COMPILED:
        _COMPILED[k_pad] = _build(k_pad)
    return _COMPILED[k_pad]


def _tile_pf(a, p=128):
    """[P*t, f...] -> contiguous [p, t, f...] partition-major tiling."""
    t = a.shape[0] // p
    return np.ascontiguousarray(
        a.reshape(t, p, *a.shape[1:]).swapaxes(0, 1))


def _prep_core_inputs(x, attention_mask, Wq, bq, Wk, bk, Wv, bv, Wo):
    """Host-side shard prep. Returns (in_maps, k_pad)."""
    x = np.asarray(x, np.float32)
    mask = np.asarray(attention_mask, bool)
    idxs = [np.nonzero(mask[b])[0] for b in range(BATCH)]
    ke_max = max(1, max(len(i) for i in idxs))
    k_pad = 384 * ((ke_max + 383) // 384)
    if k_pad > SEQ:
        k_pad = SEQ
    KC = 512 if k_pad % 512 == 0 else 384
    NKC = k_pad // KC
    KT_N = k_pad // 128

    consts = np.zeros(256, np.float32)
    consts[0:128] = 1.0

    in_maps = []
    for b in range(BATCH):
        xT = x[b].T                                  # [D, S] view
        # xq: [qc, p, dt, 512]
        xq = np.ascontiguousarray(
            xT.reshape(8, 128, 4, 512).transpose(2, 1, 0, 3))
        idx = idxs[b]
        ke = len(idx)
        if ke > k_pad:
            idx = idx[:k_pad]
            ke = k_pad
        xkT = np.zeros((D_MODEL, k_pad), np.float32)
        xkT[:, :ke] = x[b][idx].T
        # xk: [kc, p, dt, KC]
        xk = np.ascontiguousarray(
            xkT.reshape(8, 128, NKC, KC).transpose(2, 1, 0, 3))
        maskb = np.zeros(k_pad, np.float32)
        maskb[ke:] = NEG
        mb_t = _tile_pf(maskb)                       # [128, KT_N]
        for g in range(2):
            fs = slice(g * FH, (g + 1) * FH)
            # Wv/bv padded with a ones column per head: the V-projection
            # matmul then produces [V_h | ones] directly (col = 0*x + 1.0).
            Wv_aug = np.zeros((D_MODEL, HPC * 65), np.float32)
            bv_aug = np.zeros(HPC * 65, np.float32)
            for h in range(HPC):
                Wv_aug[:, h * 65:h * 65 + 64] = Wv[:, g * FH + h * 64:
                                                   g * FH + (h + 1) * 64]
                bv_aug[h * 65:h * 65 + 64] = bv[g * FH + h * 64:
                                                g * FH + (h + 1) * 64]
                bv_aug[h * 65 + 64] = 1.0
            in_maps.append({
                "xq": xq,
                "xk": xk,
                "Wq": _tile_pf(np.asarray(Wq[:, fs], np.float32)),
                "Wk": _tile_pf(np.asarray(Wk[:, fs], np.float32)),
                "Wv": _tile_pf(Wv_aug),
                "Wo": _tile_pf(np.asarray(Wo[fs, :], np.float32)),
                "bq": _tile_pf(np.asarray(bq[fs], np.float32)),
                "bk": _tile_pf(np.asarray(bk[fs], np.float32)),
                "bv": bv_aug,
                "maskb": mb_t,
                "consts": consts,
                "zpad": np.zeros(4 * k_pad, np.float32),
            })
    return in_maps, k_pad


def kernel(x, attention_mask, Wq, bq, Wk, bk, Wv, bv, Wo, bo):
    global last_results
    from concourse.bass_utils import run_bass_kernel_spmd

    in_maps, k_pad = _prep_core_inputs(x, attention_mask, Wq, bq, Wk, bk, Wv, bv, Wo)
    nc = _get_compiled(k_pad)
    res = run_bass_kernel_spmd(nc, in_maps, core_ids=list(range(N_CORES)))
    last_results = res

    bo = np.asarray(bo, np.float32)
    out = np.empty((BATCH, SEQ, D_MODEL), np.float32)
    for b in range(BATCH):
        out[b] = res.results[2 * b]["out"] + res.results[2 * b + 1]["out"] + bo
    return out


# revision 17
# speedup vs baseline: 1.4620x; 1.1861x over previous
"""Multi-head attention kernel for 8 Trainium2 NeuronCores.

Problem: B=4, S=2048, D=1024, H=16, Dh=64 MHA with key-side boolean mask.

Sharding: core c handles (batch b = c//2, head-half g = c%2, 8 heads each).
QKV are column-parallel, the output projection is row-parallel (Megatron
style); the host sums the two partial output projections per batch and adds
the output bias.

Host-side preprocessing (pure data marshalling, exact):
  - All inputs are pre-tiled into DMA-native layouts (partition-major,
    contiguous per partition) so each dma_start lowers to large linear
    descriptors instead of thousands of 2KB strided reads.
  - x is transposed per batch (the PE contracts over the partition dim, so
    x^T is required for every projection).
  - Keys with mask=False contribute exactly zero after softmax, so the host
    gathers only the unmasked keys (padded to a multiple of 384 with zero
    rows whose exp-bias is -1e30 => exp == 0 exactly). Exact, and cuts
    score/exp/attn-V work roughly in half.

On-core dataflow (all matmuls in float32r):
  xT --(Wk,Wv)--> KT[f,k] (zero-padded per head to K=128), V[k,f] (+biases)
  xT --(Wq)--> QT[f,q]
  scores^T[k,q] = [KT_h ; 0]^T x QT_pair   (K=128 full array; the zero rows
                                            kill the other head's features)
  E = exp(scores*0.125 + maskbias[k])      (one ScalarE pass, mask fused)
  out_aug[65,q] = [V_h | ones]^T x E       (row 64 = softmax denominator)
  attnT[f,q] = out_aug[0:64] * bcast(1/den)  (K=1 ones matmul broadcast)
  out[s,D] = attnT^T x Wo                  (partial; host adds pair + bo)
"""

import os
import numpy as np

os.environ.setdefault("MYCRO_LOCAL_CACHE", "1")

D_MODEL = 1024
N_HEADS = 16
D_HEAD = 64
BATCH = 4
SEQ = 2048
N_CORES = 8
FH = 512          # features per core (8 heads x 64)
HPC = 8           # heads per core
NEG = -1.0e30     # additive bias for padded/masked keys; exp -> 0 exactly

_COMPILED = {}    # k_pad -> nc
last_results = None  # BassKernelResults of the most recent run (for test.py)


def _build(k_pad):
    """Emit + compile the per-core bass kernel for a given padded key count."""
    import concourse.bacc as bacc
    import concourse.bass as bass
    import concourse.tile as tile
    from concourse import mybir

    f32 = mybir.dt.float32
    f32r = mybir.dt.float32r
    KT_N = k_pad // 128                     # number of 128-key tiles
    KC = 512 if k_pad % 512 == 0 else 384   # key-side chunk (fp32r needs N>=256)
    assert k_pad % KC == 0 and KC % 128 == 0
    NKC = k_pad // KC

    nc = bacc.Bacc("TRN2", target_bir_lowering=False, debug=False,
                   num_devices=N_CORES)

    # all pre-tiled on host into DMA-native layouts
    dxq = nc.dram_tensor("xq", [4, 128, 8, 512], f32r, kind="ExternalInput")
    dxk = nc.dram_tensor("xk", [NKC, 128, 8, KC], f32r, kind="ExternalInput")
    dWq = nc.dram_tensor("Wq", [128, 8, FH], f32r, kind="ExternalInput")
    dWk = nc.dram_tensor("Wk", [128, 8, FH], f32r, kind="ExternalInput")
    dWv = nc.dram_tensor("Wv", [128, 8, HPC * 65], f32r, kind="ExternalInput")
    dWo = nc.dram_tensor("Wo", [128, 4, D_MODEL], f32r, kind="ExternalInput")
    dbc = nc.dram_tensor("bcst", [128, 8 + KT_N], f32, kind="ExternalInput")
    dbv = nc.dram_tensor("bv", [HPC * 65], f32r, kind="ExternalInput")
    dcst = nc.dram_tensor("consts", [256], f32r, kind="ExternalInput")  # ones|zeros
    dzp = nc.dram_tensor("zpad", [4 * k_pad], f32r, kind="ExternalInput")
    dout = nc.dram_tensor("out", [SEQ, D_MODEL], f32, kind="ExternalOutput")

    EXP = mybir.ActivationFunctionType.Exp
    IDn = mybir.ActivationFunctionType.Identity

    with tile.TileContext(nc) as tc:
        with tc.tile_pool(name="persist", bufs=1) as pers:
            # ---- constants in SBUF ----
            bc = pers.tile([128, 8 + KT_N], f32, tag="bcst")
            nc.sync.dma_start(out=bc, in_=dbc.ap())
            bq = bc[:, 0:4]
            bk = bc[:, 4:8]
            mb = bc[:, 8:8 + KT_N]
            bv_row = pers.tile([1, HPC * 65], f32r, tag="bvr")
            nc.sync.dma_start(out=bv_row, in_=dbv.ap()[None, :])
            ones_t = pers.tile([1, 128], f32r, tag="ones")
            nc.sync.dma_start(out=ones_t, in_=dcst.ap()[None, 0:128])
            ones64 = ones_t[:, 0:64]
            ones128 = ones_t[:, :]

            # ---- persistent activations ----
            QT = pers.tile([128, 4, SEQ], f32r, tag="QT")        # [f, q]
            # zero-padded per-head score weights: KTe rows 0:64 = even head,
            # rows 64:128 = 0; KTo rows 0:64 = 0, rows 64:128 = odd head.
            KTe = pers.tile([128, 4, k_pad], f32r, tag="KTe")
            KTo = pers.tile([128, 4, k_pad], f32r, tag="KTo")
            Vau = pers.tile([128, KT_N, HPC, 65], f32r, tag="Vau")

            zin = bass.AP(tensor=dzp.ap().tensor, offset=0,
                          ap=[[0, 64], [1, 4 * k_pad]])
            nc.sync.dma_start(
                out=KTo[0:64, :, :].rearrange("p a k -> p (a k)"), in_=zin)

            # ================= projections =================
            wtq_cm = tc.tile_pool(name="wtq", bufs=1)
            wtq = wtq_cm.__enter__()
            wq = wtq.tile([128, 8, FH], f32r, tag="wq")
            nc.sync.dma_start(out=wq, in_=dWq.ap())
            ppool_cm = tc.tile_pool(name="pp", bufs=4, space="PSUM")
            ppool = ppool_cm.__enter__()

            # ----- K side (KT, V) -----
            with tc.tile_pool(name="wtk", bufs=1) as wtk, \
                 tc.tile_pool(name="xk", bufs=2) as xkp:
                pk = ppool
                wk = wtk.tile([128, 8, FH], f32r, tag="wk")
                nc.sync.dma_start(out=wk, in_=dWk.ap())
                wv = wtk.tile([128, 8, HPC * 65], f32r, tag="wv")
                nc.sync.dma_start(out=wv, in_=dWv.ap())
                for kc in range(NKC):
                    xk_t = xkp.tile([128, 8, KC], f32r, tag="xk")
                    nc.sync.dma_start(out=xk_t, in_=dxk.ap()[kc])
                    for ft in range(4):
                        ps = pk.tile([128, KC], f32, tag="pk")
                        for dt in range(8):
                            nc.tensor.matmul(
                                ps,
                                lhsT=wk[:, dt, ft * 128:(ft + 1) * 128],
                                rhs=xk_t[:, dt, :],
                                start=(dt == 0), stop=(dt == 7))
                        ks = slice(kc * KC, (kc + 1) * KC)
                        nc.scalar.activation(KTe[:, ft, ks], ps, IDn,
                                             bias=bk[:, ft:ft + 1])
                    for kb in range(KC // 128):
                        kg = kc * (KC // 128) + kb
                        ps = pk.tile([128, HPC * 65], f32, tag="pk")
                        for dt in range(8):
                            nc.tensor.matmul(
                                ps[:, 0:512],
                                lhsT=xk_t[:, dt, kb * 128:(kb + 1) * 128],
                                rhs=wv[:, dt, 0:512],
                                start=(dt == 0), stop=False)
                            nc.tensor.matmul(
                                ps[:, 512:520],
                                lhsT=xk_t[:, dt, kb * 128:(kb + 1) * 128],
                                rhs=wv[:, dt, 512:520],
                                start=(dt == 0), stop=False)
                        nc.tensor.matmul(ps[:, 0:512], lhsT=ones128,
                                         rhs=bv_row[:, 0:512],
                                         start=False, stop=True)
                        nc.tensor.matmul(ps[:, 512:520], lhsT=ones128,
                                         rhs=bv_row[:, 512:520],
                                         start=False, stop=True)
                        nc.scalar.copy(Vau[:, kg, :, :], ps)

            # finish zero-padded score weights: KTo upper half is a copy of
            # the drained KTe upper half; then KTe upper half becomes zero.
            nc.vector.tensor_copy(
                KTo[64:128, :, :].rearrange("p a k -> p (a k)"),
                KTe[64:128, :, :].rearrange("p a k -> p (a k)"))
            nc.sync.dma_start(
                out=KTe[64:128, :, :].rearrange("p a k -> p (a k)"), in_=zin)

            # ----- Q side (QT) -----
            with tc.tile_pool(name="xq", bufs=2) as xqp:
                pq = ppool
                for qc in range(4):
                    xq_t = xqp.tile([128, 8, 512], f32r, tag="xq")
                    nc.sync.dma_start(out=xq_t, in_=dxq.ap()[qc])
                    for ft in range(4):
                        ps = pq.tile([128, 512], f32, tag="pk")
                        for dt in range(8):
                            nc.tensor.matmul(
                                ps,
                                lhsT=wq[:, dt, ft * 128:(ft + 1) * 128],
                                rhs=xq_t[:, dt, :],
                                start=(dt == 0), stop=(dt == 7))
                        nc.scalar.activation(QT[:, ft, qc * 512:(qc + 1) * 512],
                                             ps, IDn, bias=bq[:, ft:ft + 1])

            ppool_cm.__exit__(None, None, None)
            wtq_cm.__exit__(None, None, None)

            # ================= attention core =================
            att2_cm = tc.tile_pool(name="att2", bufs=1)
            att2 = att2_cm.__enter__()
            attnT = att2.tile([128, 4, SEQ], f32r, tag="attnT")  # [f, q]
            wo = att2.tile([128, 4, D_MODEL], f32r, tag="wo")
            nc.sync.dma_start(out=wo, in_=dWo.ap())
            with tc.tile_pool(name="et", bufs=4) as etp, \
                 tc.tile_pool(name="ua", bufs=2) as uap, \
                 tc.tile_pool(name="rp", bufs=2) as rpp, \
                 tc.tile_pool(name="sp", bufs=2, space="PSUM") as sp, \
                 tc.tile_pool(name="av", bufs=2, space="PSUM") as avp:
                for t in range(4):          # head pair (heads 2t, 2t+1)
                    for qh in range(2):     # query half (1024 queries)
                        q0 = qh * 1024
                        avA = avp.tile([65, 1024], f32, tag="av")
                        avB = avp.tile([65, 1024], f32, tag="av")
                        for kt in range(KT_N):
                            kts = slice(kt * 128, (kt + 1) * 128)
                            sA = sp.tile([128, 1024], f32, tag="s")
                            sB = sp.tile([128, 1024], f32, tag="s")
                            for h in range(2):
                                c0, c1 = q0 + h * 512, q0 + (h + 1) * 512
                                nc.tensor.matmul(
                                    sA[:, h * 512:(h + 1) * 512],
                                    lhsT=KTe[:, t, kts], rhs=QT[:, t, c0:c1],
                                    start=True, stop=True)
                                nc.tensor.matmul(
                                    sB[:, h * 512:(h + 1) * 512],
                                    lhsT=KTo[:, t, kts], rhs=QT[:, t, c0:c1],
                                    start=True, stop=True)
                            eA = etp.tile([128, 1024], f32r, tag="et")
                            nc.scalar.activation(eA, sA, EXP,
                                                 bias=mb[:, kt:kt + 1], scale=0.125)
                            eB = etp.tile([128, 1024], f32r, tag="et")
                            nc.scalar.activation(eB, sB, EXP,
                                                 bias=mb[:, kt:kt + 1], scale=0.125)
                            for h in range(2):
                                cs = slice(h * 512, (h + 1) * 512)
                                nc.tensor.matmul(
                                    avA[:, cs], lhsT=Vau[:, kt, 2 * t, :],
                                    rhs=eA[:, cs],
                                    start=(kt == 0), stop=(kt == KT_N - 1))
                                nc.tensor.matmul(
                                    avB[:, cs], lhsT=Vau[:, kt, 2 * t + 1, :],
                                    rhs=eB[:, cs],
                                    start=(kt == 0), stop=(kt == KT_N - 1))
                        # normalize: attnT = out_aug[0:64] * bcast(1/den)
                        rA = rpp.tile([1, 1024], f32r, tag="r")
                        rB = rpp.tile([1, 1024], f32r, tag="r")
                        with nc.allow_low_precision(reason="fp32r matmul operand"):
                            nc.vector.reciprocal(rA, avA[64:65, :])
                            nc.vector.reciprocal(rB, avB[64:65, :])
                        bcA = sp.tile([64, 1024], f32, tag="s")
                        bcB = sp.tile([64, 1024], f32, tag="s")
                        for h in range(2):
                            cs = slice(h * 512, (h + 1) * 512)
                            nc.tensor.matmul(bcA[:, cs], lhsT=ones64,
                                             rhs=rA[:, cs], start=True, stop=True)
                            nc.tensor.matmul(bcB[:, cs], lhsT=ones64,
                                             rhs=rB[:, cs], start=True, stop=True)
                        # DVE reads at most one PSUM operand: stage out_aug's
                        # attn rows through SBUF, then multiply.
                        uA = uap.tile([64, 1024], f32, tag="ua")
                        nc.scalar.copy(uA, avA[0:64, :])
                        uB = uap.tile([64, 1024], f32, tag="ua")
                        nc.scalar.copy(uB, avB[0:64, :])
                        nc.vector.tensor_mul(attnT[0:64, t, q0:q0 + 1024],
                                             uA, bcA)
                        nc.vector.tensor_mul(attnT[64:128, t, q0:q0 + 1024],
                                             uB, bcB)

            # ================= output projection (partial) =================
            with tc.tile_pool(name="op", bufs=2, space="PSUM") as opp, \
                 tc.tile_pool(name="ot", bufs=3) as otp:
                for st in range(16):
                    ps = opp.tile([128, D_MODEL], f32, tag="op")
                    for ft in range(4):
                        for dh in range(2):
                            nc.tensor.matmul(
                                ps[:, dh * 512:(dh + 1) * 512],
                                lhsT=attnT[:, ft, st * 128:(st + 1) * 128],
                                rhs=wo[:, ft, dh * 512:(dh + 1) * 512],
                                start=(ft == 0), stop=(ft == 3))
                    ot = otp.tile([128, D_MODEL], f32, tag="ot")
                    nc.scalar.copy(ot, ps)
                    nc.sync.dma_start(out=dout.ap()[st * 128:(st + 1) * 128, :], in_=ot)
            att2_cm.__exit__(None, None, None)

    nc.compile()
    return nc


def _get_compiled(k_pad):
    if k_pad not in _COMPILED:
        _COMPILED[k_pad] = _build(k_pad)
    return _COMPILED[k_pad]


def _tile_pf(a, p=128):
    """[P*t, f...] -> contiguous [p, t, f...] partition-major tiling."""
    t = a.shape[0] // p
    return np.ascontiguousarray(
        a.reshape(t, p, *a.shape[1:]).swapaxes(0, 1))


def _prep_core_inputs(x, attention_mask, Wq, bq, Wk, bk, Wv, bv, Wo):
    """Host-side shard prep. Returns (in_maps, k_pad)."""
    x = np.asarray(x, np.float32)
    mask = np.asarray(attention_mask, bool)
    idxs = [np.nonzero(mask[b])[0] for b in range(BATCH)]
    ke_max = max(1, max(len(i) for i in idxs))
    k_pad = 384 * ((ke_max + 383) // 384)
    if k_pad > SEQ:
        k_pad = SEQ
    KC = 512 if k_pad % 512 == 0 else 384
    NKC = k_pad // KC
    KT_N = k_pad // 128

    consts = np.zeros(256, np.float32)
    consts[0:128] = 1.0

    in_maps = []
    for b in range(BATCH):
        xT = x[b].T                                  # [D, S] view
        # xq: [qc, p, dt, 512]
        xq = np.ascontiguousarray(
            xT.reshape(8, 128, 4, 512).transpose(2, 1, 0, 3))
        idx = idxs[b]
        ke = len(idx)
        if ke > k_pad:
            idx = idx[:k_pad]
            ke = k_pad
        xkT = np.zeros((D_MODEL, k_pad), np.float32)
        xkT[:, :ke] = x[b][idx].T
        # xk: [kc, p, dt, KC]
        xk = np.ascontiguousarray(
            xkT.reshape(8, 128, NKC, KC).transpose(2, 1, 0, 3))
        maskb = np.zeros(k_pad, np.float32)
        maskb[ke:] = NEG
        mb_t = _tile_pf(maskb)                       # [128, KT_N]
        KT_N = k_pad // 128
        for g in range(2):
            fs = slice(g * FH, (g + 1) * FH)
            # Wv/bv padded with a ones column per head: the V-projection
            # matmul then produces [V_h | ones] directly (col = 0*x + 1.0).
            Wv_aug = np.zeros((D_MODEL, HPC * 65), np.float32)
            bv_aug = np.zeros(HPC * 65, np.float32)
            for h in range(HPC):
                Wv_aug[:, h * 65:h * 65 + 64] = Wv[:, g * FH + h * 64:
                                                   g * FH + (h + 1) * 64]
                bv_aug[h * 65:h * 65 + 64] = bv[g * FH + h * 64:
                                                g * FH + (h + 1) * 64]
                bv_aug[h * 65 + 64] = 1.0
            in_maps.append({
                "xq": xq,
                "xk": xk,
                "Wq": _tile_pf(np.asarray(Wq[:, fs], np.float32)),
                "Wk": _tile_pf(np.asarray(Wk[:, fs], np.float32)),
                "Wv": _tile_pf(Wv_aug),
                "Wo": _tile_pf(np.asarray(Wo[fs, :], np.float32)),
                "bcst": np.concatenate(
                    [_tile_pf(np.asarray(bq[fs], np.float32)),
                     _tile_pf(np.asarray(bk[fs], np.float32)),
                     mb_t], axis=1).astype(np.float32),
                "bv": bv_aug,
                "consts": consts,
                "zpad": np.zeros(4 * k_pad, np.float32),
            })
    return in_maps, k_pad


def kernel(x, attention_mask, Wq, bq, Wk, bk, Wv, bv, Wo, bo):
    global last_results
    from concourse.bass_utils import run_bass_kernel_spmd

    in_maps, k_pad = _prep_core_inputs(x, attention_mask, Wq, bq, Wk, bk, Wv, bv, Wo)
    nc = _get_compiled(k_pad)
    res = run_bass_kernel_spmd(nc, in_maps, core_ids=list(range(N_CORES)))
    last_results = res

    bo = np.asarray(bo, np.float32)
    out = np.empty((BATCH, SEQ, D_MODEL), np.float32)
    for b in range(BATCH):
        out[b] = res.results[2 * b]["out"] + res.results[2 * b + 1]["out"] + bo
    return out
